# revision 19
# baseline (speedup 1.0000x reference)
"""Multi-relation GAT layer on 8 Trainium2 NeuronCores.

Sharding: cores 0-3 own batch 0, cores 4-7 own batch 1; within a batch each
core owns a quarter of the destination rows (IS=512) for ALL R relations, so
the relation-mean, residual and LayerNorm stay core-local (no collectives).

Host precomputes the dense projections (Wh = H@W, attention dots es/ed) and
all scalar exponentials; the device does the heavy O(R*Hh*N*IS) masked
attention with two elementwise passes per tile.

Algebra: scores are rank-1 before the leaky-relu (s[i,j] = es_i + ed_j);
softmax over j is invariant to per-i scaling, and per-j scaling folds into
the aggregation weights (linear over j). Dividing exp(lrelu(s)) by exp(es_i)
(per-i) and t_j = exp(0.2*ed_j) (per-j, folded into Wh) leaves

    u[j,i] = m[j,i] * max(invtau_j, g_i)
      invtau = exp(0.8*ed), g = exp(-0.8*es)      (host-precomputed)
      W~h = Wh * t,  ones column = R * t          (host-folded)

Device per j-tile/head:
    v = g_b max invtau_j        tensor_scalar, one AP scalar (2x DVE mode)
    u = v * m                   tensor_tensor batched over (j-tile, head);
                                mask broadcast over heads via stride-0
                                middle AP dim keeps the 2x mode
    agg^T[f,i] += W~h^T u       PE accumulation matmuls; ones column gives
                                R*denominator directly
Epilogue per (r,h): PE transposes to [i,f] layout, DVE reciprocal, ACT does
the rec-scaled copy, DVE accumulates over r; then residual + LayerNorm.
The per-r epilogue is software-pipelined one iteration behind the hot loop.
(The Pool engine is left idle on purpose: concurrent Pool tensor ops stall
DVE ops via shared SBUF ports, netting zero.)
"""

import sys

sys.path.insert(0, "/opt/trn_rl_repo")

import numpy as np

R, B, N, D, Hh, hd = 3, 2, 2048, 128, 4, 32
NCORES = 8
NQ = 4  # i-shards per batch
IS = N // NQ  # 512 dst rows per core
NIT = IS // 128  # 4 partition tiles of dst rows
NT = N // 128  # 16 j tiles
LN_EPS = 1e-5
HW = Hh * 33  # 132 packed Wh cols per j-tile (32 wh + 1 ones per head)
GJT = 2  # j-tiles per v/u buffer

_CACHE = {}


def _build_program():
    import concourse.bass as bass
    import concourse.mybir as mybir
    import concourse.tile as tile
    from concourse import bacc
    from concourse.masks import make_identity
    from contextlib import ExitStack

    f32 = mybir.dt.float32
    f16 = mybir.dt.float16
    Alu = mybir.AluOpType
    Act = mybir.ActivationFunctionType

    nc = bacc.Bacc("TRN2", target_bir_lowering=False, debug=False)
    mq = nc.declare_dram_parameter("mq", [R, 128, NT * IS], f16, isOutput=False)
    whp = nc.declare_dram_parameter("whp", [R, 128, NT * HW], f16, isOutput=False)
    gb4 = nc.declare_dram_parameter("gb4", [R, 128, Hh * IS], f16, isOutput=False)
    itc = nc.declare_dram_parameter("itc", [R, 128, NT * Hh], f32, isOutput=False)
    hres = nc.declare_dram_parameter("hres", [NIT, 128, D], f32, isOutput=False)
    gmb = nc.declare_dram_parameter("gmb", [2, 128, D], f32, isOutput=False)
    out = nc.declare_dram_parameter("out", [NIT, 128, D], f32, isOutput=True)

    with ExitStack() as ctx:
        tc = ctx.enter_context(tile.TileContext(nc))
        const = ctx.enter_context(tc.tile_pool(name="const", bufs=1))
        mq_pool = ctx.enter_context(tc.tile_pool(name="mq", bufs=2))
        v_pool = ctx.enter_context(tc.tile_pool(name="v", bufs=3))
        u_pool = ctx.enter_context(tc.tile_pool(name="u", bufs=3))
        aggsb_pool = ctx.enter_context(tc.tile_pool(name="aggsb", bufs=8))
        small = ctx.enter_context(tc.tile_pool(name="small", bufs=8))
        epi_pool = ctx.enter_context(tc.tile_pool(name="epi", bufs=2))
        psum_agg = ctx.enter_context(tc.tile_pool(name="pagg", bufs=1, space="PSUM"))
        psum_tp = ctx.enter_context(tc.tile_pool(name="ptp", bufs=4, space="PSUM"))

        # ---- constants / per-relation operands ----
        ident = const.tile([128, 128], f32, tag="ident")
        make_identity(nc, ident[:])

        whp_sb, gb4_sb, itc_sb = [], [], []
        mq_pre = {}
        for r in range(R):
            if r < 1:  # prefetch first mask early so r=0 tts don't stall
                m = mq_pool.tile([128, NT * IS], f16, tag="mq")
                nc.sync.dma_start(m[:], mq[r])
                mq_pre[r] = m
            g = const.tile([128, Hh * IS], f16, tag=f"gb4{r}")
            nc.sync.dma_start(g[:], gb4[r])
            gb4_sb.append(g)
            q = const.tile([128, NT * Hh], f32, tag=f"itc{r}")
            nc.sync.dma_start(q[:], itc[r])
            itc_sb.append(q)
            w = const.tile([128, NT * HW], f16, tag=f"whp{r}")
            nc.sync.dma_start(w[:], whp[r])
            whp_sb.append(w)

        hres_sb, acc = [], []
        for t in range(NIT):
            hh = const.tile([128, D], f32, tag=f"hres{t}")
            nc.sync.dma_start(hh[:], hres[t])
            hres_sb.append(hh)
            acc_t = const.tile([128, D], f32, tag=f"acc{t}", name=f"acc{t}")
            acc.append(acc_t)
        gam = const.tile([128, D], f32, tag="gam")
        nc.sync.dma_start(gam[:], gmb[0])
        bet = const.tile([128, D], f32, tag="bet")
        nc.sync.dma_start(bet[:], gmb[1])
        eps_b = const.tile([128, 1], f32, tag="eps_b")
        nc.gpsimd.memset(eps_b[:], LN_EPS)

        # ---- hot loop over relations ----
        pend = []

        def _emit_epi(item):
            er, asbs = item
            for h in range(Hh):
                for it in range(NIT):
                    tp = psum_tp.tile([128, 33], f32, tag="tp")
                    nc.tensor.transpose(
                        tp[:], asbs[h][:, it * 128 : (it + 1) * 128], ident[:33, :33]
                    )
                    rec = small.tile([128, 1], f32, tag="rec")
                    nc.vector.reciprocal(rec[:], tp[:, 32:33])
                    dst = acc[it][:, h * hd : (h + 1) * hd]
                    if er == 0:
                        # acc = rec * numer directly (ACT scaled copy)
                        nc.scalar.activation(dst, tp[:, 0:32], Act.Copy, scale=rec[:])
                    else:
                        contrib = small.tile([128, hd], f32, tag="contrib")
                        nc.scalar.activation(
                            contrib[:], tp[:, 0:32], Act.Copy, scale=rec[:]
                        )
                        nc.vector.tensor_add(dst, dst, contrib[:])

        for r in range(R):
            if r in mq_pre:
                m_sb = mq_pre[r]
            else:
                m_sb = mq_pool.tile([128, NT * IS], f16, tag="mq")
                nc.sync.dma_start(m_sb[:], mq[r])

            aggp = [
                psum_agg.tile([33, IS], f32, tag=f"agg{h}", name=f"agg{h}")
                for h in range(Hh)
            ]

            for g in range(NT // GJT):
                v = v_pool.tile([128, GJT * Hh * IS], f16, tag="v")
                for jl in range(GJT):
                    jt = g * GJT + jl
                    for h in range(Hh):
                        # v = g_i max invtau_j
                        nc.vector.tensor_scalar(
                            out=v[:, (jl * Hh + h) * IS : (jl * Hh + h + 1) * IS],
                            in0=gb4_sb[r][:, h * IS : (h + 1) * IS],
                            scalar1=itc_sb[r][:, jt * Hh + h : jt * Hh + h + 1],
                            scalar2=None,
                            op0=Alu.max,
                        )
                u = u_pool.tile([128, GJT * Hh * IS], f16, tag="u")
                m3 = m_sb[:, g * GJT * IS : (g + 1) * GJT * IS].rearrange(
                    "p (a i) -> p a i", a=GJT
                )
                nc.vector.tensor_mul(
                    u[:].rearrange("p (a h i) -> p a h i", a=GJT, h=Hh),
                    v[:].rearrange("p (a h i) -> p a h i", a=GJT, h=Hh),
                    m3[:, :, None, :].broadcast_to([128, GJT, Hh, IS]),
                )
                for jl in range(GJT):
                    jt = g * GJT + jl
                    for h in range(Hh):
                        nc.tensor.matmul(
                            aggp[h][:, :],
                            lhsT=whp_sb[r][:, jt * HW + h * 33 : jt * HW + (h + 1) * 33],
                            rhs=u[:, (jl * Hh + h) * IS : (jl * Hh + h + 1) * IS],
                            start=(jt == 0),
                            stop=(jt == NT - 1),
                        )

            # ---- drain PSUM quickly (frees agg banks for next r's chains) ----
            asbs = []
            for h in range(Hh):
                asb = aggsb_pool.tile([33, IS], f32, tag="aggsb")
                nc.scalar.copy(asb[:], aggp[h][:])
                asbs.append(asb)
            pend.append((r, asbs))

            # ---- deferred epilogue of the previous relation ----
            if len(pend) > 1:
                _emit_epi(pend.pop(0))

        _emit_epi(pend.pop(0))

        # ---- epilogue: residual + LayerNorm ----
        for t in range(NIT):
            x = epi_pool.tile([128, D], f32, tag="x")
            nc.vector.tensor_add(x[:], acc[t][:], hres_sb[t][:])
            mu = small.tile([128, 1], f32, tag="mu")
            nc.vector.reduce_sum(out=mu[:], in_=x[:], axis=mybir.AxisListType.X)
            nc.vector.tensor_scalar_mul(mu[:], mu[:], 1.0 / D)
            xc = epi_pool.tile([128, D], f32, tag="xc")
            nc.vector.tensor_scalar(
                out=xc[:], in0=x[:], scalar1=mu[:], scalar2=None, op0=Alu.subtract
            )
            sq = epi_pool.tile([128, D], f32, tag="sq")
            nc.vector.tensor_mul(sq[:], xc[:], xc[:])
            vs = small.tile([128, 1], f32, tag="vs")
            nc.vector.reduce_sum(out=vs[:], in_=sq[:], axis=mybir.AxisListType.X)
            nc.vector.tensor_scalar_mul(vs[:], vs[:], 1.0 / D)
            std = small.tile([128, 1], f32, tag="std")
            nc.scalar.activation(std[:], vs[:], Act.Sqrt, bias=eps_b[:])
            rstd = small.tile([128, 1], f32, tag="rstd")
            nc.vector.reciprocal(rstd[:], std[:])
            xn = epi_pool.tile([128, D], f32, tag="xn")
            nc.vector.tensor_scalar(
                out=xn[:], in0=xc[:], scalar1=rstd[:], scalar2=None, op0=Alu.mult
            )
            xg = epi_pool.tile([128, D], f32, tag="xg")
            nc.vector.tensor_mul(xg[:], xn[:], gam[:])
            xo = epi_pool.tile([128, D], f32, tag="xo")
            nc.vector.tensor_add(xo[:], xg[:], bet[:])
            nc.sync.dma_start(out[t], xo[:])

    nc.compile()
    return nc


def _host_pack(H, A, W, a_src, a_dst, ln_gamma, ln_beta):
    H = np.asarray(H, np.float32)
    A = np.asarray(A)
    W = np.asarray(W, np.float32)
    a_src = np.asarray(a_src, np.float32)
    a_dst = np.asarray(a_dst, np.float32)
    ln_gamma = np.asarray(ln_gamma, np.float32)
    ln_beta = np.asarray(ln_beta, np.float32)

    Hm = H.reshape(B * N, D)
    # Wh[r,b,n,h,f]
    Wh = np.empty((R, B, N, Hh, hd), np.float32)
    for r in range(R):
        for h in range(Hh):
            Wh[r, :, :, h, :] = (Hm @ W[r, h]).reshape(B, N, hd)
    es = np.einsum("rbnhf,rhf->rbhn", Wh, a_src)  # [R,B,Hh,N]
    ed = np.einsum("rbnhf,rhf->rbhn", Wh, a_dst)

    t_f = np.exp(0.2 * ed)  # [R,B,Hh,N]  (j-indexed)
    invtau = np.exp(0.8 * ed).astype(np.float32)
    g_all = np.exp(-0.8 * es).astype(np.float16)  # [R,B,Hh,N]  (i-indexed)

    # packed W~h = Wh * t plus R*t ones column, per batch: [B, R, 128, NT*132] f16
    whp = np.empty((R, B, NT, 128, Hh, 33), np.float32)
    tj = t_f.transpose(0, 1, 3, 2).reshape(R, B, NT, 128, Hh)
    whp[..., :32] = Wh.reshape(R, B, NT, 128, Hh, hd) * tj[..., None]
    whp[..., 32] = R * tj
    whp = (
        whp.reshape(R, B, NT, 128, HW)
        .transpose(1, 0, 3, 2, 4)
        .reshape(B, R, 128, NT * HW)
        .astype(np.float16)
    )
    whp = np.ascontiguousarray(whp)

    # invtau scalar columns [B, R, 128, NT*Hh] f32
    itc = np.ascontiguousarray(
        invtau.reshape(R, B, Hh, NT, 128).transpose(1, 0, 4, 3, 2)
    ).reshape(B, R, 128, NT * Hh)
    itc = np.ascontiguousarray(itc)

    # raw 0/1 mask, transposed: [R,B,j,i_all] fp16
    At = A.transpose(0, 1, 3, 2)
    mq_full = At.astype(np.float16).reshape(R, B, NT, 128, N)

    gmbase = np.stack(
        [
            np.broadcast_to(ln_gamma, (128, D)),
            np.broadcast_to(ln_beta, (128, D)),
        ]
    ).astype(np.float32)
    gmbase = np.ascontiguousarray(gmbase)

    in_maps = []
    for c in range(NCORES):
        b, q = divmod(c, NQ)
        i0 = q * IS
        mq_c = np.ascontiguousarray(
            mq_full[:, b, :, :, i0 : i0 + IS].transpose(0, 2, 1, 3)
        ).reshape(R, 128, NT * IS)
        g_c = g_all[:, b, :, i0 : i0 + IS].reshape(R, Hh * IS)
        g_c = np.ascontiguousarray(np.broadcast_to(g_c[:, None, :], (R, 128, Hh * IS)))
        hres_c = np.ascontiguousarray(H[b, i0 : i0 + IS, :]).reshape(NIT, 128, D)
        in_maps.append(
            {
                "mq": mq_c,
                "whp": whp[b],
                "gb4": g_c,
                "itc": itc[b],
                "hres": hres_c,
                "gmb": gmbase,
            }
        )
    return in_maps


def kernel(H, A, W, a_src, a_dst, ln_gamma, ln_beta):
    from concourse.bass_utils import run_bass_kernel_spmd

    if "nc" not in _CACHE:
        _CACHE["nc"] = _build_program()
    nc = _CACHE["nc"]

    in_maps = _host_pack(H, A, W, a_src, a_dst, ln_gamma, ln_beta)
    res = run_bass_kernel_spmd(nc, in_maps, list(range(NCORES)))

    full = np.empty((B, N, D), np.float32)
    for c in range(NCORES):
        b, q = divmod(c, NQ)
        o = np.asarray(res.results[c]["out"], np.float32).reshape(IS, D)
        full[b, q * IS : (q + 1) * IS, :] = o
    return full


# revision 21
# speedup vs baseline: 1.2334x; 1.2334x over previous
"""Multi-relation GAT layer on 8 Trainium2 NeuronCores.

Sharding: cores 0-3 own batch 0, cores 4-7 own batch 1; within a batch each
core owns a quarter of the destination rows (IS=512) for ALL R relations, so
the relation-mean, residual and LayerNorm stay core-local (no collectives).

Host precomputes the dense projections (Wh = H@W, attention dots es/ed) and
all scalar exponentials; the device does the heavy O(R*Hh*N*IS) masked
attention with two elementwise passes per tile.

Algebra: scores are rank-1 before the leaky-relu (s[i,j] = es_i + ed_j);
softmax over j is invariant to per-i scaling, and per-j scaling folds into
the aggregation weights (linear over j). Dividing exp(lrelu(s)) by exp(es_i)
(per-i) and t_j = exp(0.2*ed_j) (per-j, folded into Wh) leaves

    u[j,i] = m[j,i] * max(invtau_j, g_i)
      invtau = exp(0.8*ed), g = exp(-0.8*es)      (host-precomputed)
      W~h = Wh * t,  ones column = R * t          (host-folded)

Device per j-tile/head:
    v = g_b max invtau_j        tensor_scalar, one AP scalar (2x DVE mode)
    u = v * m                   tensor_tensor batched over (j-tile, head);
                                mask broadcast over heads via stride-0
                                middle AP dim keeps the 2x mode
    agg^T[f,i] += W~h^T u       PE accumulation matmuls; ones column gives
                                R*denominator directly
Epilogue per (r,h): PE transposes to [i,f] layout, DVE reciprocal, ACT does
the rec-scaled copy, DVE accumulates over r; then residual + LayerNorm.
The per-r epilogue is software-pipelined one iteration behind the hot loop.
(The Pool engine is left idle on purpose: concurrent Pool tensor ops stall
DVE ops via shared SBUF ports, netting zero.)
"""

import sys

sys.path.insert(0, "/opt/trn_rl_repo")

import numpy as np

R, B, N, D, Hh, hd = 3, 2, 2048, 128, 4, 32
NCORES = 8
NQ = 4  # i-shards per batch
IS = N // NQ  # 512 dst rows per core
NIT = IS // 128  # 4 partition tiles of dst rows
NT = N // 128  # 16 j tiles
LN_EPS = 1e-5
HW = Hh * 33  # 132 packed Wh cols per j-tile (32 wh + 1 ones per head)
GJT = 2  # j-tiles per v/u buffer

_CACHE = {}


def _build_program():
    import concourse.bass as bass
    import concourse.mybir as mybir
    import concourse.tile as tile
    from concourse import bacc
    from concourse.masks import make_identity
    from contextlib import ExitStack

    f32 = mybir.dt.float32
    f16 = mybir.dt.float16
    Alu = mybir.AluOpType
    Act = mybir.ActivationFunctionType

    nc = bacc.Bacc("TRN2", target_bir_lowering=False, debug=False)
    mq = nc.declare_dram_parameter("mq", [R, 128, NT * IS], f16, isOutput=False)
    whp = nc.declare_dram_parameter("whp", [R, 128, NT * HW], f16, isOutput=False)
    gb4 = nc.declare_dram_parameter("gb4", [R, 128, Hh * IS], f16, isOutput=False)
    itc = nc.declare_dram_parameter("itc", [R, 128, NT * Hh], f32, isOutput=False)
    hres = nc.declare_dram_parameter("hres", [NIT, 128, D], f32, isOutput=False)
    gmb = nc.declare_dram_parameter("gmb", [2, 128, D], f32, isOutput=False)
    out = nc.declare_dram_parameter("out", [NIT, 128, D], f32, isOutput=True)

    with ExitStack() as ctx:
        tc = ctx.enter_context(tile.TileContext(nc))
        const = ctx.enter_context(tc.tile_pool(name="const", bufs=1))
        mq_pool = ctx.enter_context(tc.tile_pool(name="mq", bufs=2))
        v_pool = ctx.enter_context(tc.tile_pool(name="v", bufs=3))
        u_pool = ctx.enter_context(tc.tile_pool(name="u", bufs=3))
        aggsb_pool = ctx.enter_context(tc.tile_pool(name="aggsb", bufs=8))
        small = ctx.enter_context(tc.tile_pool(name="small", bufs=8))
        epi_pool = ctx.enter_context(tc.tile_pool(name="epi", bufs=2))
        psum_agg = ctx.enter_context(tc.tile_pool(name="pagg", bufs=1, space="PSUM"))
        psum_tp = ctx.enter_context(tc.tile_pool(name="ptp", bufs=4, space="PSUM"))

        # ---- constants / per-relation operands ----
        ident = const.tile([128, 128], f32, tag="ident")
        make_identity(nc, ident[:])

        whp_sb, gb4_sb, itc_sb = [], [], []
        mq_pre = {}
        for r in range(R):
            g = const.tile([128, Hh * IS], f16, tag=f"gb4{r}")
            nc.sync.dma_start(g[:], gb4[r])
            gb4_sb.append(g)
            q = const.tile([128, NT * Hh], f32, tag=f"itc{r}")
            nc.sync.dma_start(q[:], itc[r])
            itc_sb.append(q)
            if r < 1:  # prefetch first mask early, in chunks (group order)
                m = mq_pool.tile([128, NT * IS], f16, tag="mq")
                CH = NT * IS // 4
                for k in range(4):
                    nc.sync.dma_start(
                        m[:, k * CH : (k + 1) * CH],
                        mq[r][:, k * CH : (k + 1) * CH],
                    )
                mq_pre[r] = m
            w = const.tile([128, NT * HW], f16, tag=f"whp{r}")
            nc.sync.dma_start(w[:], whp[r])
            whp_sb.append(w)

        hres_sb, acc = [], []
        for t in range(NIT):
            hh = const.tile([128, D], f32, tag=f"hres{t}")
            nc.sync.dma_start(hh[:], hres[t])
            hres_sb.append(hh)
            acc_t = const.tile([128, D], f32, tag=f"acc{t}", name=f"acc{t}")
            acc.append(acc_t)
        gam = const.tile([128, D], f32, tag="gam")
        nc.sync.dma_start(gam[:], gmb[0])
        bet = const.tile([128, D], f32, tag="bet")
        nc.sync.dma_start(bet[:], gmb[1])
        eps_b = const.tile([128, 1], f32, tag="eps_b")
        nc.gpsimd.memset(eps_b[:], LN_EPS)

        # ---- hot loop over relations ----
        pend = []

        def _emit_epi(item):
            er, asbs = item
            for it in range(NIT):
                if er > 0:
                    contrib = small.tile([128, D], f32, tag="contrib", name="contrib")
                else:
                    contrib = None
                for h in range(Hh):
                    tp = psum_tp.tile([128, 33], f32, tag="tp")
                    nc.tensor.transpose(
                        tp[:], asbs[h][:, it * 128 : (it + 1) * 128], ident[:33, :33]
                    )
                    den = small.tile([128, 1], f32, tag="den")
                    nc.scalar.copy(den[:], tp[:, 32:33])
                    rec = small.tile([128, 1], f32, tag="rec")
                    nc.vector.reciprocal(rec[:], den[:])
                    dst = (
                        acc[it][:, h * hd : (h + 1) * hd]
                        if er == 0
                        else contrib[:, h * hd : (h + 1) * hd]
                    )
                    # dst = rec * numer (ACT scaled copy)
                    nc.scalar.activation(dst, tp[:, 0:32], Act.Copy, scale=rec[:])
                if er > 0:
                    nc.vector.tensor_add(acc[it][:], acc[it][:], contrib[:])

        for r in range(R):
            if r in mq_pre:
                m_sb = mq_pre[r]
            else:
                m_sb = mq_pool.tile([128, NT * IS], f16, tag="mq")
                nc.sync.dma_start(m_sb[:], mq[r])

            aggp = [
                psum_agg.tile([33, IS], f32, tag=f"agg{h}", name=f"agg{h}")
                for h in range(Hh)
            ]

            for g in range(NT // GJT):
                v = v_pool.tile([128, GJT * Hh * IS], f16, tag="v")
                for jl in range(GJT):
                    jt = g * GJT + jl
                    for h in range(Hh):
                        # v = g_i max invtau_j
                        nc.vector.tensor_scalar(
                            out=v[:, (jl * Hh + h) * IS : (jl * Hh + h + 1) * IS],
                            in0=gb4_sb[r][:, h * IS : (h + 1) * IS],
                            scalar1=itc_sb[r][:, jt * Hh + h : jt * Hh + h + 1],
                            scalar2=None,
                            op0=Alu.max,
                        )
                u = u_pool.tile([128, GJT * Hh * IS], f16, tag="u")
                m3 = m_sb[:, g * GJT * IS : (g + 1) * GJT * IS].rearrange(
                    "p (a i) -> p a i", a=GJT
                )
                nc.vector.tensor_mul(
                    u[:].rearrange("p (a h i) -> p a h i", a=GJT, h=Hh),
                    v[:].rearrange("p (a h i) -> p a h i", a=GJT, h=Hh),
                    m3[:, :, None, :].broadcast_to([128, GJT, Hh, IS]),
                )
                for jl in range(GJT):
                    jt = g * GJT + jl
                    for h in range(Hh):
                        nc.tensor.matmul(
                            aggp[h][:, :],
                            lhsT=whp_sb[r][:, jt * HW + h * 33 : jt * HW + (h + 1) * 33],
                            rhs=u[:, (jl * Hh + h) * IS : (jl * Hh + h + 1) * IS],
                            start=(jt == 0),
                            stop=(jt == NT - 1),
                        )

            # ---- drain PSUM quickly (frees agg banks for next r's chains) ----
            asbs = []
            for h in range(Hh):
                asb = aggsb_pool.tile([33, IS], f32, tag="aggsb")
                nc.scalar.copy(asb[:], aggp[h][:])
                asbs.append(asb)
            pend.append((r, asbs))

            # ---- deferred epilogue of the previous relation ----
            if len(pend) > 1:
                _emit_epi(pend.pop(0))

        _emit_epi(pend.pop(0))

        # ---- epilogue: residual + LayerNorm ----
        for t in range(NIT):
            x = epi_pool.tile([128, D], f32, tag="x")
            nc.vector.tensor_add(x[:], acc[t][:], hres_sb[t][:])
            mu = small.tile([128, 1], f32, tag="mu")
            nc.vector.reduce_sum(out=mu[:], in_=x[:], axis=mybir.AxisListType.X)
            nc.vector.tensor_scalar_mul(mu[:], mu[:], 1.0 / D)
            xc = epi_pool.tile([128, D], f32, tag="xc")
            nc.vector.tensor_scalar(
                out=xc[:], in0=x[:], scalar1=mu[:], scalar2=None, op0=Alu.subtract
            )
            sq = epi_pool.tile([128, D], f32, tag="sq")
            nc.vector.tensor_mul(sq[:], xc[:], xc[:])
            vs = small.tile([128, 1], f32, tag="vs")
            nc.vector.reduce_sum(out=vs[:], in_=sq[:], axis=mybir.AxisListType.X)
            nc.vector.tensor_scalar_mul(vs[:], vs[:], 1.0 / D)
            std = small.tile([128, 1], f32, tag="std")
            nc.scalar.activation(std[:], vs[:], Act.Sqrt, bias=eps_b[:])
            rstd = small.tile([128, 1], f32, tag="rstd")
            nc.vector.reciprocal(rstd[:], std[:])
            xn = epi_pool.tile([128, D], f32, tag="xn")
            nc.vector.tensor_scalar(
                out=xn[:], in0=xc[:], scalar1=rstd[:], scalar2=None, op0=Alu.mult
            )
            xg = epi_pool.tile([128, D], f32, tag="xg")
            nc.vector.tensor_mul(xg[:], xn[:], gam[:])
            xo = epi_pool.tile([128, D], f32, tag="xo")
            nc.vector.tensor_add(xo[:], xg[:], bet[:])
            nc.sync.dma_start(out[t], xo[:])

    nc.compile()
    return nc


def _host_pack(H, A, W, a_src, a_dst, ln_gamma, ln_beta):
    H = np.asarray(H, np.float32)
    A = np.asarray(A)
    W = np.asarray(W, np.float32)
    a_src = np.asarray(a_src, np.float32)
    a_dst = np.asarray(a_dst, np.float32)
    ln_gamma = np.asarray(ln_gamma, np.float32)
    ln_beta = np.asarray(ln_beta, np.float32)

    Hm = H.reshape(B * N, D)
    # Wh[r,b,n,h,f]
    Wh = np.empty((R, B, N, Hh, hd), np.float32)
    for r in range(R):
        for h in range(Hh):
            Wh[r, :, :, h, :] = (Hm @ W[r, h]).reshape(B, N, hd)
    es = np.einsum("rbnhf,rhf->rbhn", Wh, a_src)  # [R,B,Hh,N]
    ed = np.einsum("rbnhf,rhf->rbhn", Wh, a_dst)

    t_f = np.exp(0.2 * ed)  # [R,B,Hh,N]  (j-indexed)
    invtau = np.exp(0.8 * ed).astype(np.float32)
    g_all = np.exp(-0.8 * es).astype(np.float16)  # [R,B,Hh,N]  (i-indexed)

    # packed W~h = Wh * t plus R*t ones column, per batch: [B, R, 128, NT*132] f16
    whp = np.empty((R, B, NT, 128, Hh, 33), np.float32)
    tj = t_f.transpose(0, 1, 3, 2).reshape(R, B, NT, 128, Hh)
    whp[..., :32] = Wh.reshape(R, B, NT, 128, Hh, hd) * tj[..., None]
    whp[..., 32] = R * tj
    whp = (
        whp.reshape(R, B, NT, 128, HW)
        .transpose(1, 0, 3, 2, 4)
        .reshape(B, R, 128, NT * HW)
        .astype(np.float16)
    )
    whp = np.ascontiguousarray(whp)

    # invtau scalar columns [B, R, 128, NT*Hh] f32
    itc = np.ascontiguousarray(
        invtau.reshape(R, B, Hh, NT, 128).transpose(1, 0, 4, 3, 2)
    ).reshape(B, R, 128, NT * Hh)
    itc = np.ascontiguousarray(itc)

    # raw 0/1 mask, transposed: [R,B,j,i_all] fp16
    At = A.transpose(0, 1, 3, 2)
    mq_full = At.astype(np.float16).reshape(R, B, NT, 128, N)

    gmbase = np.stack(
        [
            np.broadcast_to(ln_gamma, (128, D)),
            np.broadcast_to(ln_beta, (128, D)),
        ]
    ).astype(np.float32)
    gmbase = np.ascontiguousarray(gmbase)

    in_maps = []
    for c in range(NCORES):
        b, q = divmod(c, NQ)
        i0 = q * IS
        mq_c = np.ascontiguousarray(
            mq_full[:, b, :, :, i0 : i0 + IS].transpose(0, 2, 1, 3)
        ).reshape(R, 128, NT * IS)
        g_c = g_all[:, b, :, i0 : i0 + IS].reshape(R, Hh * IS)
        g_c = np.ascontiguousarray(np.broadcast_to(g_c[:, None, :], (R, 128, Hh * IS)))
        hres_c = np.ascontiguousarray(H[b, i0 : i0 + IS, :]).reshape(NIT, 128, D)
        in_maps.append(
            {
                "mq": mq_c,
                "whp": whp[b],
                "gb4": g_c,
                "itc": itc[b],
                "hres": hres_c,
                "gmb": gmbase,
            }
        )
    return in_maps


def kernel(H, A, W, a_src, a_dst, ln_gamma, ln_beta):
    from concourse.bass_utils import run_bass_kernel_spmd

    if "nc" not in _CACHE:
        _CACHE["nc"] = _build_program()
    nc = _CACHE["nc"]

    in_maps = _host_pack(H, A, W, a_src, a_dst, ln_gamma, ln_beta)
    res = run_bass_kernel_spmd(nc, in_maps, list(range(NCORES)))

    full = np.empty((B, N, D), np.float32)
    for c in range(NCORES):
        b, q = divmod(c, NQ)
        o = np.asarray(res.results[c]["out"], np.float32).reshape(IS, D)
        full[b, q * IS : (q + 1) * IS, :] = o
    return full
